# revision 5
# baseline (speedup 1.0000x reference)
"""Trainium2 Bass kernel for nn_BottleneckResAtnMHSA.

Reference computation (per image, C=128, N=1024 spatial tokens):
  x1 = silu(bn1(w1 @ x))                      # [128, 1024]
  q/k/v = w{q,k,v} @ x1 + b{q,k,v}            # [128, 1024]
  logits = q^T k + pos^T q                    # [1024, 1024]
  att = softmax(logits, axis=-1)
  out = v @ att^T                             # [128, 1024]
  y = x + silu(bn2(w2 @ out))                 # [256, 1024]

Kernel strategy (data-parallel over batch, 4 images / NeuronCore):
  * BN scales folded into conv weights host-side; BN shifts applied as
    per-partition activation biases.
  * logits are computed transposed, attT[j,i] = sum_c B[c,j] A[c,i] with
    B = [k; q], A = [q; pos]: one K=256 PSUM accumulation, and j lands on
    partitions so that att^T is directly consumable by the v-matmul.
  * softmax runs without max subtraction (|logits| < 40 for this model's
    data distribution, exp stays comfortably inside fp32 range).
  * column sums of exp(attT) come from an all-ones [128,128] stationary
    matmul which also broadcasts the denominator across partitions.
  * v bias is folded into the cv2 bias (b2 += w2' @ bv) so the
    unnormalized v-matmul output only needs a reciprocal multiply.
"""

import numpy as np

N_CORES = 8
B_PER_CORE = 4
C = 128
CIN = 256
N = 1024
EPS = 1e-5

_CACHE = {}


def _build_program(native_silu=True):
    import concourse.bacc as bacc
    import concourse.mybir as mybir
    import concourse.tile as tile

    f32 = mybir.dt.float32
    AF = mybir.ActivationFunctionType
    ALU = mybir.AluOpType

    nc = bacc.Bacc("TRN2", target_bir_lowering=False, debug=False,
                   num_devices=N_CORES)

    xs = nc.dram_tensor("xs", [B_PER_CORE, CIN, N], f32, kind="ExternalInput").ap()
    w1t = nc.dram_tensor("w1t", [CIN, C], f32, kind="ExternalInput").ap()
    t1c = nc.dram_tensor("t1c", [C, 1], f32, kind="ExternalInput").ap()
    wqt = nc.dram_tensor("wqt", [C, C], f32, kind="ExternalInput").ap()
    bqc = nc.dram_tensor("bqc", [C, 1], f32, kind="ExternalInput").ap()
    wkt = nc.dram_tensor("wkt", [C, C], f32, kind="ExternalInput").ap()
    bkc = nc.dram_tensor("bkc", [C, 1], f32, kind="ExternalInput").ap()
    wvt = nc.dram_tensor("wvt", [C, C], f32, kind="ExternalInput").ap()
    pos = nc.dram_tensor("pos", [C, N], f32, kind="ExternalInput").ap()
    w2t = nc.dram_tensor("w2t", [C, CIN], f32, kind="ExternalInput").ap()
    b2c = nc.dram_tensor("b2c", [C, 2], f32, kind="ExternalInput").ap()
    ys = nc.dram_tensor("ys", [B_PER_CORE, CIN, N], f32, kind="ExternalOutput").ap()

    with tile.TileContext(nc) as tc:
        with (
            tc.tile_pool(name="consts", bufs=1) as consts,
            tc.tile_pool(name="act", bufs=1) as act,
            tc.tile_pool(name="psum", bufs=1, space="PSUM") as psum,
        ):
            # ---- load constants once ----
            w1t_sb = consts.tile([128, 2, C], f32, tag="w1t")
            nc.sync.dma_start(w1t_sb, w1t.rearrange("(k p) m -> p k m", p=128))
            wqt_sb = consts.tile([128, C], f32, tag="wqt")
            nc.sync.dma_start(wqt_sb, wqt)
            wkt_sb = consts.tile([128, C], f32, tag="wkt")
            nc.sync.dma_start(wkt_sb, wkt)
            wvt_sb = consts.tile([128, C], f32, tag="wvt")
            nc.sync.dma_start(wvt_sb, wvt)
            pos_sb = consts.tile([128, N], f32, tag="pos")
            nc.sync.dma_start(pos_sb, pos)
            w2t_sb = consts.tile([128, CIN], f32, tag="w2t")
            nc.sync.dma_start(w2t_sb, w2t)
            t1c_sb = consts.tile([128, 1], f32, tag="t1c")
            nc.sync.dma_start(t1c_sb, t1c)
            bqc_sb = consts.tile([128, 1], f32, tag="bqc")
            nc.sync.dma_start(bqc_sb, bqc)
            bkc_sb = consts.tile([128, 1], f32, tag="bkc")
            nc.sync.dma_start(bkc_sb, bkc)
            b2c_sb = consts.tile([128, 2], f32, tag="b2c")
            nc.sync.dma_start(b2c_sb, b2c)
            ones_sb = consts.tile([128, 128], f32, tag="ones")
            nc.vector.memset(ones_sb, 1.0)

            HALF = [slice(0, 512), slice(512, 1024)]

            def silu(out_sb, ps, bias_col):
                """out = silu(ps + bias) with bias broadcast per partition."""
                if native_silu:
                    nc.scalar.activation(out_sb, ps, AF.Silu, bias=bias_col)
                else:
                    # CoreSim has no Silu LUT: silu(z) = z * sigmoid(z)
                    nc.scalar.activation(out_sb, ps, AF.Sigmoid, bias=bias_col)
                    nc.vector.scalar_tensor_tensor(
                        out_sb, ps, bias_col, out_sb,
                        op0=ALU.add, op1=ALU.mult)

            for b in range(B_PER_CORE):
                # ---- load x[b]: [256, 1024] as [p, k, n] ----
                x_sb = act.tile([128, 2, N], f32, tag="x", bufs=2)
                nc.sync.dma_start(x_sb, xs[b].rearrange("(k p) n -> p k n", p=128))

                # ---- cv1: x1 = silu(w1' @ x + t1) ----
                ps_x1 = psum.tile([128, N], f32, tag="mm", bufs=2)
                for k in range(2):
                    for h in HALF:
                        nc.tensor.matmul(ps_x1[:, h], w1t_sb[:, k, :],
                                         x_sb[:, k, h],
                                         start=(k == 0), stop=(k == 1))
                x1_sb = act.tile([128, N], f32, tag="x1", bufs=2)
                silu(x1_sb, ps_x1, t1c_sb[:, 0:1])

                # ---- q, k: [c, i] layout ----
                ps_q = psum.tile([128, N], f32, tag="mm", bufs=2)
                for h in HALF:
                    nc.tensor.matmul(ps_q[:, h], wqt_sb, x1_sb[:, h],
                                     start=True, stop=True)
                q_sb = act.tile([128, N], f32, tag="q", bufs=2)
                nc.vector.tensor_scalar_add(q_sb, ps_q, bqc_sb)

                ps_k = psum.tile([128, N], f32, tag="mm", bufs=2)
                for h in HALF:
                    nc.tensor.matmul(ps_k[:, h], wkt_sb, x1_sb[:, h],
                                     start=True, stop=True)
                k_sb = act.tile([128, N], f32, tag="k", bufs=2)
                nc.vector.tensor_scalar_add(k_sb, ps_k, bkc_sb)

                # ---- vT: [i, c] layout (bias folded into cv2) ----
                ps_vt = psum.tile([128, N], f32, tag="mm", bufs=2)
                for it in range(8):
                    sl = slice(it * 128, (it + 1) * 128)
                    nc.tensor.matmul(ps_vt[:, sl], x1_sb[:, sl], wvt_sb,
                                     start=True, stop=True)
                vt_sb = act.tile([128, N], f32, tag="vt", bufs=2)
                nc.vector.tensor_copy(vt_sb, ps_vt)

                # ---- attention ----
                ps_s = psum.tile([128, N], f32, tag="s", bufs=1)
                ps_o = psum.tile([128, N], f32, tag="o", bufs=1)
                e_sb = act.tile([128, 8 * N], f32, tag="e", bufs=1)
                for jt in range(8):
                    sl = slice(jt * 128, (jt + 1) * 128)
                    ps_att = psum.tile([128, N], f32, tag="mm", bufs=2)
                    # attT[j, i] = sum_c k[c,j] q[c,i] + sum_c q[c,j] pos[c,i]
                    for h in HALF:
                        nc.tensor.matmul(ps_att[:, h], k_sb[:, sl], q_sb[:, h],
                                         start=True, stop=False)
                    for h in HALF:
                        nc.tensor.matmul(ps_att[:, h], q_sb[:, sl], pos_sb[:, h],
                                         start=False, stop=True)
                    ej = e_sb[:, jt * N:(jt + 1) * N]
                    nc.scalar.activation(ej, ps_att, AF.Exp)
                    # denominator: all-ones stationary -> column sums
                    # broadcast over all 128 partitions
                    for h in HALF:
                        nc.tensor.matmul(ps_s[:, h], ones_sb, ej[:, h],
                                         start=(jt == 0), stop=(jt == 7),
                                         skip_group_check=True)
                    # numerator: out[c, i] += sum_j v[c, j] e[j, i]
                    for h in HALF:
                        nc.tensor.matmul(ps_o[:, h], vt_sb[:, sl], ej[:, h],
                                         start=(jt == 0), stop=(jt == 7),
                                         skip_group_check=True)

                recip_sb = act.tile([128, N], f32, tag="recip", bufs=2)
                nc.vector.reciprocal(recip_sb, ps_s)
                outn_sb = act.tile([128, N], f32, tag="outn", bufs=2)
                nc.vector.tensor_mul(outn_sb, ps_o, recip_sb)

                # ---- cv2 + residual ----
                for mt in range(2):
                    sl = slice(mt * 128, (mt + 1) * 128)
                    ps_y = psum.tile([128, N], f32, tag="mm", bufs=2)
                    for h in HALF:
                        nc.tensor.matmul(ps_y[:, h], w2t_sb[:, sl], outn_sb[:, h],
                                         start=True, stop=True)
                    y_sb = act.tile([128, N], f32, tag="y", bufs=2)
                    silu(y_sb, ps_y, b2c_sb[:, mt:mt + 1])
                    res_sb = act.tile([128, N], f32, tag="res", bufs=2)
                    nc.vector.tensor_add(res_sb, y_sb, x_sb[:, mt, :])
                    nc.sync.dma_start(ys[b, sl, :], res_sb)

    nc.compile()
    return nc


def _prepare_params(w1, bn1_g, bn1_b, bn1_m, bn1_v, wq, bq, wk, bk, wv, bv,
                    rel_h, rel_w, w2, bn2_g, bn2_b, bn2_m, bn2_v):
    f64 = np.float64
    s1 = bn1_g.astype(f64) / np.sqrt(bn1_v.astype(f64) + EPS)
    w1p = w1.astype(f64) * s1[:, None]
    t1 = bn1_b.astype(f64) - bn1_m.astype(f64) * s1
    s2 = bn2_g.astype(f64) / np.sqrt(bn2_v.astype(f64) + EPS)
    w2p = w2.astype(f64) * s2[:, None]
    t2 = bn2_b.astype(f64) - bn2_m.astype(f64) * s2
    b2 = t2 + w2p @ bv.astype(f64)
    posv = (rel_h.astype(f64) + rel_w.astype(f64)).reshape(C, N)
    f32 = np.float32
    return {
        "w1t": np.ascontiguousarray(w1p.T, dtype=f32),
        "t1c": np.ascontiguousarray(t1[:, None], dtype=f32),
        "wqt": np.ascontiguousarray(wq.T, dtype=f32),
        "bqc": np.ascontiguousarray(bq[:, None], dtype=f32),
        "wkt": np.ascontiguousarray(wk.T, dtype=f32),
        "bkc": np.ascontiguousarray(bk[:, None], dtype=f32),
        "wvt": np.ascontiguousarray(wv.T, dtype=f32),
        "pos": np.ascontiguousarray(posv, dtype=f32),
        "w2t": np.ascontiguousarray(w2p.T, dtype=f32),
        "b2c": np.ascontiguousarray(
            b2.astype(f32).reshape(2, 128).T, dtype=f32),
    }


def get_program(native_silu=True):
    key = ("nc", native_silu)
    if key not in _CACHE:
        _CACHE[key] = _build_program(native_silu)
    return _CACHE[key]


def make_in_maps(x, params):
    B = x.shape[0]
    per = B // N_CORES
    xr = np.ascontiguousarray(x.reshape(B, CIN, N), dtype=np.float32)
    return [
        {"xs": xr[c * per:(c + 1) * per], **params}
        for c in range(N_CORES)
    ]


def kernel(x, w1, bn1_g, bn1_b, bn1_m, bn1_v, wq, bq, wk, bk, wv, bv,
           rel_h, rel_w, w2, bn2_g, bn2_b, bn2_m, bn2_v):
    from concourse.bass_utils import run_bass_kernel_spmd

    nc = get_program()
    params = _prepare_params(w1, bn1_g, bn1_b, bn1_m, bn1_v, wq, bq, wk, bk,
                             wv, bv, rel_h, rel_w, w2, bn2_g, bn2_b, bn2_m,
                             bn2_v)
    in_maps = make_in_maps(x, params)
    res = run_bass_kernel_spmd(nc, in_maps, core_ids=list(range(N_CORES)))
    out = np.concatenate([res.results[c]["ys"] for c in range(N_CORES)], axis=0)
    return np.ascontiguousarray(out.reshape(32, CIN, 32, 32), dtype=np.float32)


# revision 17
# speedup vs baseline: 2.9787x; 2.9787x over previous
"""Trainium2 Bass kernel for nn_BottleneckResAtnMHSA.

Reference computation (per image, C=128, N=1024 spatial tokens):
  x1 = silu(bn1(w1 @ x))                      # [128, 1024]
  q/k/v = w{q,k,v} @ x1 + b{q,k,v}            # [128, 1024]
  logits = q^T k + pos^T q                    # [1024, 1024]
  att = softmax(logits, axis=-1)
  out = v @ att^T                             # [128, 1024]
  y = x + silu(bn2(w2 @ out))                 # [256, 1024]

Kernel strategy (data-parallel over batch, 4 images / NeuronCore):
  * BN scales folded into conv weights host-side; BN shifts applied as
    per-partition activation biases.
  * all matmuls run as float32r (TF32-class PE mode, 4x the fp32 rate;
    fp32 data unchanged in memory, PSUM accumulation stays fp32).
  * logits are computed transposed, attT[j,i] = sum_c B[c,j] A[c,i] with
    B = [k; q], A = [q; pos]: one K=256 PSUM accumulation, and j lands on
    partitions so att^T is directly consumable by the v-matmul.
  * softmax runs without max subtraction (|logits| < 40 for this model's
    data distribution, exp stays comfortably inside fp32 range).
  * column sums of exp(attT) come from an all-ones [128,128] stationary
    matmul which also broadcasts the denominator across partitions.
  * v bias is folded into the cv2 bias (b2 += w2' @ bv) so the
    unnormalized v-matmul output only needs a reciprocal multiply.
  * emission order A(0) B(0) | A(1) C(0) B(1) | ... | C(3) with
    half-tile (512-col) ACT/DVE granularity keeps every engine streaming;
    DMAs all ride the sync/HWDGE queue in need-order.
"""

import numpy as np

N_CORES = 8
B_PER_CORE = 4
C = 128
CIN = 256
N = 1024
EPS = 1e-5

_CACHE = {}


def _build_program(native_silu=True, use_f32r=True):
    import concourse.bacc as bacc
    import concourse.mybir as mybir
    import concourse.tile as tile

    f32 = mybir.dt.float32
    AF = mybir.ActivationFunctionType
    ALU = mybir.AluOpType

    nc = bacc.Bacc("TRN2", target_bir_lowering=False, debug=False,
                   num_devices=N_CORES)

    xs = nc.dram_tensor("xs", [B_PER_CORE, CIN, N], f32, kind="ExternalInput").ap()
    w1t = nc.dram_tensor("w1t", [CIN, C], f32, kind="ExternalInput").ap()
    wqt = nc.dram_tensor("wqt", [C, C], f32, kind="ExternalInput").ap()
    wkt = nc.dram_tensor("wkt", [C, C], f32, kind="ExternalInput").ap()
    wvt = nc.dram_tensor("wvt", [C, C], f32, kind="ExternalInput").ap()
    pos = nc.dram_tensor("pos", [C, N], f32, kind="ExternalInput").ap()
    w2t = nc.dram_tensor("w2t", [C, CIN], f32, kind="ExternalInput").ap()
    bpk = nc.dram_tensor("bpk", [C, 5], f32, kind="ExternalInput").ap()
    onesd = nc.dram_tensor("onesd", [C, C], f32, kind="ExternalInput").ap()
    ys = nc.dram_tensor("ys", [B_PER_CORE, CIN, N], f32, kind="ExternalOutput").ap()

    HALF = [slice(0, 512), slice(512, 1024)]

    with tile.TileContext(nc) as tc:
        with (
            tc.tile_pool(name="consts", bufs=1) as consts,
            tc.tile_pool(name="act", bufs=1) as act,
            tc.tile_pool(name="psum", bufs=1, space="PSUM") as psum,
        ):
            # const tiles; DMAs are emitted in first-need order below
            bpk_sb = consts.tile([128, 5], f32, tag="bpk")
            w1t_sb = consts.tile([128, 2, C], f32, tag="w1t")
            wqt_sb = consts.tile([128, C], f32, tag="wqt")
            wkt_sb = consts.tile([128, C], f32, tag="wkt")
            # wvt duplicated along free dim: N=256 keeps float32r matmuls
            # at full rate (moving dim >= 256); half the result is unused
            wvt_sb = consts.tile([128, 2, C], f32, tag="wvt")
            pos_sb = consts.tile([128, N], f32, tag="pos")
            w2t_sb = consts.tile([128, CIN], f32, tag="w2t")
            t1c_sb = bpk_sb[:, 0:1]
            bqc_sb = bpk_sb[:, 1:2]
            bkc_sb = bpk_sb[:, 2:3]
            b2c_sb = bpk_sb[:, 3:5]

            fr = (lambda ap: ap.bitcast(mybir.dt.float32r)) if use_f32r \
                else (lambda ap: ap)
            frw = fr  # producer-side marker: walrus wants fp32r-matmul
            # operands to be written as fp32r (same 4-byte layout)

            nc.sync.dma_start(bpk_sb, bpk)
            nc.sync.dma_start(frw(w1t_sb), fr(w1t.rearrange("(k p) m -> p k m", p=128)))

            ones_sb = consts.tile([128, 128], f32, tag="ones")
            nc.sync.dma_start(frw(ones_sb), fr(onesd))
            # warm the silu LUT at t~0 so the first real silu isn't stuck
            # behind a late table load
            warm_sb = consts.tile([128, 1], f32, tag="warm")
            nc.vector.memset(warm_sb, 0.0)
            nc.scalar.activation(warm_sb, warm_sb,
                                 AF.Silu if native_silu else AF.Sigmoid)


            def mm(out, lhsT, rhs, **kw):
                nc.tensor.matmul(out, fr(lhsT), fr(rhs), **kw)

            def silu(out_sb, ps, bias_col, h, round_r=False):
                """out[:, h] = silu(ps + bias); ps is a [128,512] psum."""
                w = frw if round_r else (lambda a: a)
                if native_silu:
                    nc.scalar.activation(w(out_sb[:, h]), ps, AF.Silu,
                                         bias=bias_col)
                else:
                    # CoreSim has no Silu LUT: silu(z) = z * sigmoid(z)
                    nc.scalar.activation(out_sb[:, h], ps, AF.Sigmoid,
                                         bias=bias_col)
                    nc.vector.scalar_tensor_tensor(
                        w(out_sb[:, h]), ps, bias_col, out_sb[:, h],
                        op0=ALU.add, op1=ALU.mult)

            x_sbs, q_sbs, k_sbs, vt_sbs, outn_sbs = [], [], [], [], []

            def phase_a(b):
                x_sb = act.tile([128, 2, N], f32, tag="x", bufs=3, name=f"x{b}")
                xr = xs[b].rearrange("(k p) n -> p k n", p=128)
                for k in range(2):
                    for h in HALF:
                        nc.sync.dma_start(frw(x_sb[:, k, h]), fr(xr[:, k, h]))
                x_sbs.append(x_sb)

                x1_sb = act.tile([128, N], f32, tag="x1", bufs=2, name=f"x1_{b}")
                for hi, h in enumerate(HALF):
                    ps = psum.tile([128, 512], f32, tag="mm", bufs=4,
                                   name=f"psx1_{b}_{hi}")
                    for k in range(2):
                        mm(ps, w1t_sb[:, k, :], x_sb[:, k, h],
                           start=(k == 0), stop=(k == 1))
                    silu(x1_sb, ps, t1c_sb, h, round_r=use_f32r)
                if b == 0:
                    nc.sync.dma_start(frw(wqt_sb), fr(wqt))
                    nc.sync.dma_start(frw(wkt_sb), fr(wkt))
                    nc.sync.dma_start(frw(wvt_sb[:, 0, :]), fr(wvt))
                    nc.sync.dma_start(frw(wvt_sb[:, 1, :]), fr(wvt))

                q_sb = act.tile([128, N], f32, tag="q", bufs=2, name=f"q{b}")
                k_sb = act.tile([128, N], f32, tag="k", bufs=2, name=f"k{b}")
                for hi, h in enumerate(HALF):
                    ps = psum.tile([128, 512], f32, tag="mm", bufs=4,
                                   name=f"psq_{b}_{hi}")
                    mm(ps, wqt_sb, x1_sb[:, h], start=True, stop=True)
                    nc.vector.tensor_scalar_add(frw(q_sb[:, h]), ps, bqc_sb)
                for hi, h in enumerate(HALF):
                    ps = psum.tile([128, 512], f32, tag="mm", bufs=4,
                                   name=f"psk_{b}_{hi}")
                    mm(ps, wkt_sb, x1_sb[:, h], start=True, stop=True)
                    nc.vector.tensor_scalar_add(frw(k_sb[:, h]), ps, bkc_sb)
                q_sbs.append(q_sb)
                k_sbs.append(k_sb)

                # vT in four 2-token-tile chunks; each matmul writes [128, 256]
                # (duplicated result), DVE strided-copies out the first halves
                vt_sb = act.tile([128, N], f32, tag="vt", bufs=2, name=f"vt{b}")
                for c4 in range(4):
                    ps = psum.tile([128, 2, 2, C], f32, tag="mm", bufs=4,
                                   name=f"psvt_{b}_{c4}")
                    for it in range(2):
                        sl = slice((c4 * 2 + it) * 128, (c4 * 2 + it + 1) * 128)
                        mm(ps[:, it, :, :], x1_sb[:, sl], wvt_sb,
                           start=True, stop=True)
                    nc.vector.tensor_copy(
                        frw(vt_sb[:, c4 * 256:(c4 + 1) * 256].rearrange(
                            "p (i c) -> p i c", i=2)),
                        ps[:, :, 0, :])
                vt_sbs.append(vt_sb)
                if b == 0:
                    nc.sync.dma_start(frw(pos_sb), fr(pos))
                    nc.sync.dma_start(frw(w2t_sb), fr(w2t))

            def phase_b(b):
                # attention, software-pipelined: PE stream is att(0) att(1)
                # [s/o(0)] att(2) [s/o(1)] ... so PE never head-of-line
                # blocks on ACT's exp.
                q_sb, k_sb, vt_sb = q_sbs[b], k_sbs[b], vt_sbs[b]
                ps_s = psum.tile([128, N], f32, tag="s", bufs=1, name=f"pss_{b}")
                ps_o = psum.tile([128, N], f32, tag="o", bufs=1, name=f"pso_{b}")
                e_sb = act.tile([128, 8 * N], f32, tag="e", bufs=2, name=f"e{b}")

                # pull the exp LUT swap ahead of the first att matmul
                nc.scalar.activation(warm_sb, warm_sb, AF.Exp)

                def emit_att(jt):
                    # attT[j,i] = sum_c k[c,j] q[c,i] + sum_c q[c,j] pos[c,i]
                    # one [128,512] psum per half, each complete after 2 mms
                    sl = slice(jt * 128, (jt + 1) * 128)
                    pair = []
                    for hi, h in enumerate(HALF):
                        ps = psum.tile([128, 512], f32, tag="mm", bufs=4,
                                       name=f"psatt_{b}_{jt}_{hi}")
                        mm(ps, k_sb[:, sl], q_sb[:, h], start=True, stop=False)
                        mm(ps, q_sb[:, sl], pos_sb[:, h], start=False, stop=True)
                        pair.append(ps)
                    return pair

                ps_att = emit_att(0)
                for jt in range(8):
                    cur = ps_att
                    if jt < 7:
                        ps_att = emit_att(jt + 1)
                    sl = slice(jt * 128, (jt + 1) * 128)
                    ej = e_sb[:, jt * N:(jt + 1) * N]
                    for hi, h in enumerate(HALF):
                        nc.scalar.activation(frw(ej[:, h]), cur[hi], AF.Exp)
                        # denominator: all-ones stationary -> column sums
                        # broadcast across all 128 partitions
                        mm(ps_s[:, h], ones_sb, ej[:, h],
                           start=(jt == 0), stop=(jt == 7),
                           skip_group_check=True)
                        # numerator: out[c, i] += sum_j v[c, j] e[j, i]
                        mm(ps_o[:, h], vt_sb[:, sl], ej[:, h],
                           start=(jt == 0), stop=(jt == 7),
                           skip_group_check=True)

                recip_sb = act.tile([128, N], f32, tag="recip", bufs=2,
                                    name=f"recip{b}")
                outn_sb = act.tile([128, N], f32, tag="outn", bufs=2,
                                   name=f"outn{b}")
                for h in HALF:
                    nc.vector.reciprocal(recip_sb[:, h], ps_s[:, h])
                    nc.vector.tensor_mul(frw(outn_sb[:, h]), ps_o[:, h],
                                         recip_sb[:, h])
                outn_sbs.append(outn_sb)

            def phase_c(b):
                for mt in range(2):
                    sl = slice(mt * 128, (mt + 1) * 128)
                    y_sb = act.tile([128, N], f32, tag="y", bufs=2,
                                    name=f"y{b}_{mt}")
                    res_sb = act.tile([128, N], f32, tag="res", bufs=2,
                                      name=f"res{b}_{mt}")
                    for hi, h in enumerate(HALF):
                        ps = psum.tile([128, 512], f32, tag="mm", bufs=4,
                                       name=f"psy_{b}_{mt}_{hi}")
                        mm(ps, w2t_sb[:, sl], outn_sbs[b][:, h],
                           start=True, stop=True)
                        silu(y_sb, ps, b2c_sb[:, mt:mt + 1], h)
                        nc.vector.tensor_add(res_sb[:, h], y_sb[:, h],
                                             x_sbs[b][:, mt, h])
                        nc.sync.dma_start(ys[b, sl, h], res_sb[:, h])

            phase_a(0)
            phase_b(0)
            for b in range(1, B_PER_CORE):
                phase_a(b)
                phase_c(b - 1)
                phase_b(b)
            phase_c(B_PER_CORE - 1)

    nc.compile()
    return nc


def _prepare_params(w1, bn1_g, bn1_b, bn1_m, bn1_v, wq, bq, wk, bk, wv, bv,
                    rel_h, rel_w, w2, bn2_g, bn2_b, bn2_m, bn2_v):
    f64 = np.float64
    s1 = bn1_g.astype(f64) / np.sqrt(bn1_v.astype(f64) + EPS)
    w1p = w1.astype(f64) * s1[:, None]
    t1 = bn1_b.astype(f64) - bn1_m.astype(f64) * s1
    s2 = bn2_g.astype(f64) / np.sqrt(bn2_v.astype(f64) + EPS)
    w2p = w2.astype(f64) * s2[:, None]
    t2 = bn2_b.astype(f64) - bn2_m.astype(f64) * s2
    b2 = t2 + w2p @ bv.astype(f64)
    posv = (rel_h.astype(f64) + rel_w.astype(f64)).reshape(C, N)
    f32 = np.float32
    bpk = np.stack([t1, bq.astype(f64), bk.astype(f64),
                    b2[:128], b2[128:]], axis=1)
    return {
        "w1t": np.ascontiguousarray(w1p.T, dtype=f32),
        "wqt": np.ascontiguousarray(wq.T, dtype=f32),
        "wkt": np.ascontiguousarray(wk.T, dtype=f32),
        "wvt": np.ascontiguousarray(wv.T, dtype=f32),
        "pos": np.ascontiguousarray(posv, dtype=f32),
        "w2t": np.ascontiguousarray(w2p.T, dtype=f32),
        "bpk": np.ascontiguousarray(bpk, dtype=f32),
        "onesd": np.ones((C, C), dtype=f32),
    }


def get_program(native_silu=True, use_f32r=True):
    key = ("nc", native_silu, use_f32r)
    if key not in _CACHE:
        _CACHE[key] = _build_program(native_silu, use_f32r)
    return _CACHE[key]


def make_in_maps(x, params):
    B = x.shape[0]
    per = B // N_CORES
    xr = np.ascontiguousarray(x.reshape(B, CIN, N), dtype=np.float32)
    return [
        {"xs": xr[c * per:(c + 1) * per], **params}
        for c in range(N_CORES)
    ]


def kernel(x, w1, bn1_g, bn1_b, bn1_m, bn1_v, wq, bq, wk, bk, wv, bv,
           rel_h, rel_w, w2, bn2_g, bn2_b, bn2_m, bn2_v):
    from concourse.bass_utils import run_bass_kernel_spmd

    nc = get_program()
    params = _prepare_params(w1, bn1_g, bn1_b, bn1_m, bn1_v, wq, bq, wk, bk,
                             wv, bv, rel_h, rel_w, w2, bn2_g, bn2_b, bn2_m,
                             bn2_v)
    in_maps = make_in_maps(x, params)
    res = run_bass_kernel_spmd(nc, in_maps, core_ids=list(range(N_CORES)))
    out = np.concatenate([res.results[c]["ys"] for c in range(N_CORES)], axis=0)
    return np.ascontiguousarray(out.reshape(32, CIN, 32, 32), dtype=np.float32)


# revision 24
# speedup vs baseline: 3.0102x; 1.0106x over previous
"""Trainium2 Bass kernel for nn_BottleneckResAtnMHSA.

Reference computation (per image, C=128, N=1024 spatial tokens):
  x1 = silu(bn1(w1 @ x))                      # [128, 1024]
  q/k/v = w{q,k,v} @ x1 + b{q,k,v}            # [128, 1024]
  logits = q^T k + pos^T q                    # [1024, 1024]
  att = softmax(logits, axis=-1)
  out = v @ att^T                             # [128, 1024]
  y = x + silu(bn2(w2 @ out))                 # [256, 1024]

Kernel strategy (data-parallel over batch, 4 images / NeuronCore):
  * BN scales folded into conv weights host-side; BN shifts applied as
    per-partition activation biases.
  * all matmuls run as float32r (TF32-class PE mode, 4x the fp32 rate;
    fp32 data unchanged in memory, PSUM accumulation stays fp32).
  * logits are computed transposed, attT[j,i] = sum_c B[c,j] A[c,i] with
    B = [k; q], A = [q; pos]: one K=256 PSUM accumulation, and j lands on
    partitions so att^T is directly consumable by the v-matmul.
  * softmax runs without max subtraction (|logits| < 40 for this model's
    data distribution, exp stays comfortably inside fp32 range).
  * column sums of exp(attT) come from an all-ones [128,128] stationary
    matmul which also broadcasts the denominator across partitions.
  * v bias is folded into the cv2 bias (b2 += w2' @ bv) so the
    unnormalized v-matmul output only needs a reciprocal multiply.
  * emission order A(0) B(0) | A(1) C(0) B(1) | ... | C(3) with
    half-tile (512-col) ACT/DVE granularity keeps every engine streaming;
    DMAs all ride the sync/HWDGE queue in need-order.
"""

import numpy as np

N_CORES = 8
B_PER_CORE = 4
C = 128
CIN = 256
N = 1024
EPS = 1e-5

_CACHE = {}


def _build_program(native_silu=True, use_f32r=True):
    import concourse.bacc as bacc
    import concourse.mybir as mybir
    import concourse.tile as tile

    f32 = mybir.dt.float32
    AF = mybir.ActivationFunctionType
    ALU = mybir.AluOpType

    nc = bacc.Bacc("TRN2", target_bir_lowering=False, debug=False,
                   num_devices=N_CORES)

    xs = nc.dram_tensor("xs", [B_PER_CORE, CIN, N], f32, kind="ExternalInput").ap()
    w1t = nc.dram_tensor("w1t", [CIN, C], f32, kind="ExternalInput").ap()
    wqt = nc.dram_tensor("wqt", [C, C], f32, kind="ExternalInput").ap()
    wkt = nc.dram_tensor("wkt", [C, C], f32, kind="ExternalInput").ap()
    wvt = nc.dram_tensor("wvt", [C, C], f32, kind="ExternalInput").ap()
    pos = nc.dram_tensor("pos", [C, N], f32, kind="ExternalInput").ap()
    w2t = nc.dram_tensor("w2t", [C, CIN], f32, kind="ExternalInput").ap()
    bpk = nc.dram_tensor("bpk", [C, 5], f32, kind="ExternalInput").ap()
    onesd = nc.dram_tensor("onesd", [C, C], f32, kind="ExternalInput").ap()
    ys = nc.dram_tensor("ys", [B_PER_CORE, CIN, N], f32, kind="ExternalOutput").ap()

    HALF = [slice(0, 512), slice(512, 1024)]

    with tile.TileContext(nc) as tc:
        with (
            tc.tile_pool(name="consts", bufs=1) as consts,
            tc.tile_pool(name="act", bufs=1) as act,
            tc.tile_pool(name="psum", bufs=1, space="PSUM") as psum,
        ):
            # const tiles; DMAs are emitted in first-need order below
            bpk_sb = consts.tile([128, 5], f32, tag="bpk")
            w1t_sb = consts.tile([128, 2, C], f32, tag="w1t")
            wqt_sb = consts.tile([128, C], f32, tag="wqt")
            wkt_sb = consts.tile([128, C], f32, tag="wkt")
            # wvt duplicated along free dim: N=256 keeps float32r matmuls
            # at full rate (moving dim >= 256); half the result is unused
            wvt_sb = consts.tile([128, 2, C], f32, tag="wvt")
            pos_sb = consts.tile([128, N], f32, tag="pos")
            w2t_sb = consts.tile([128, CIN], f32, tag="w2t")
            t1c_sb = bpk_sb[:, 0:1]
            bqc_sb = bpk_sb[:, 1:2]
            bkc_sb = bpk_sb[:, 2:3]
            b2c_sb = bpk_sb[:, 3:5]

            fr = (lambda ap: ap.bitcast(mybir.dt.float32r)) if use_f32r \
                else (lambda ap: ap)
            frw = fr  # producer-side marker: walrus wants fp32r-matmul
            # operands to be written as fp32r (same 4-byte layout)


            ones_sb = consts.tile([128, 128], f32, tag="ones")
            nc.sync.dma_start(frw(ones_sb), fr(onesd))
            # warm the silu LUT at t~0 so the first real silu isn't stuck
            # behind a late table load
            warm_sb = consts.tile([128, 1], f32, tag="warm")
            nc.vector.memset(warm_sb, 0.0)
            nc.scalar.activation(warm_sb, warm_sb,
                                 AF.Silu if native_silu else AF.Sigmoid)


            def mm(out, lhsT, rhs, **kw):
                nc.tensor.matmul(out, fr(lhsT), fr(rhs), **kw)

            def silu(out_sb, ps, bias_col, h, round_r=False):
                """out[:, h] = silu(ps + bias); ps is a [128,512] psum."""
                w = frw if round_r else (lambda a: a)
                if native_silu:
                    nc.scalar.activation(w(out_sb[:, h]), ps, AF.Silu,
                                         bias=bias_col)
                else:
                    # CoreSim has no Silu LUT: silu(z) = z * sigmoid(z)
                    nc.scalar.activation(out_sb[:, h], ps, AF.Sigmoid,
                                         bias=bias_col)
                    nc.vector.scalar_tensor_tensor(
                        w(out_sb[:, h]), ps, bias_col, out_sb[:, h],
                        op0=ALU.add, op1=ALU.mult)

            x_sbs, q_sbs, k_sbs, vt_sbs, outn_sbs = [], [], [], [], []

            def phase_a(b):
                x_sb = act.tile([128, 2, N], f32, tag="x", bufs=4, name=f"x{b}")
                xr = xs[b].rearrange("(k p) n -> p k n", p=128)
                if b == 0:
                    # need-ordered cold start: first x quarter, then the
                    # cv1 weights + biases, then the rest of x
                    nc.sync.dma_start(frw(x_sb[:, 0, HALF[0]]), fr(xr[:, 0, HALF[0]]))
                    nc.sync.dma_start(
                        frw(w1t_sb), fr(w1t.rearrange("(k p) m -> p k m", p=128)))
                    nc.sync.dma_start(bpk_sb, bpk)
                    nc.sync.dma_start(frw(x_sb[:, 0, HALF[1]]), fr(xr[:, 0, HALF[1]]))
                    for h in HALF:
                        nc.sync.dma_start(frw(x_sb[:, 1, h]), fr(xr[:, 1, h]))
                else:
                    for k in range(2):
                        for h in HALF:
                            nc.sync.dma_start(frw(x_sb[:, k, h]), fr(xr[:, k, h]))
                x_sbs.append(x_sb)

                x1_sb = act.tile([128, N], f32, tag="x1", bufs=2, name=f"x1_{b}")
                for hi, h in enumerate(HALF):
                    ps = psum.tile([128, 512], f32, tag="mm", bufs=4,
                                   name=f"psx1_{b}_{hi}")
                    for k in range(2):
                        mm(ps, w1t_sb[:, k, :], x_sb[:, k, h],
                           start=(k == 0), stop=(k == 1))
                    silu(x1_sb, ps, t1c_sb, h, round_r=use_f32r)
                if b == 0:
                    nc.sync.dma_start(frw(wqt_sb), fr(wqt))
                    nc.sync.dma_start(frw(wkt_sb), fr(wkt))
                    nc.sync.dma_start(frw(wvt_sb[:, 0, :]), fr(wvt))
                    nc.sync.dma_start(frw(wvt_sb[:, 1, :]), fr(wvt))

                q_sb = act.tile([128, N], f32, tag="q", bufs=4, name=f"q{b}")
                k_sb = act.tile([128, N], f32, tag="k", bufs=4, name=f"k{b}")
                for hi, h in enumerate(HALF):
                    ps = psum.tile([128, 512], f32, tag="mm", bufs=4,
                                   name=f"psq_{b}_{hi}")
                    mm(ps, wqt_sb, x1_sb[:, h], start=True, stop=True)
                    nc.scalar.activation(frw(q_sb[:, h]), ps, AF.Identity,
                                         bias=bqc_sb)
                for hi, h in enumerate(HALF):
                    ps = psum.tile([128, 512], f32, tag="mm", bufs=4,
                                   name=f"psk_{b}_{hi}")
                    mm(ps, wkt_sb, x1_sb[:, h], start=True, stop=True)
                    nc.vector.tensor_scalar_add(frw(k_sb[:, h]), ps, bkc_sb)
                q_sbs.append(q_sb)
                k_sbs.append(k_sb)

                # vT in four 2-token-tile chunks; each matmul writes [128, 256]
                # (duplicated result), DVE strided-copies out the first halves
                vt_sb = act.tile([128, N], f32, tag="vt", bufs=4, name=f"vt{b}")
                for c4 in range(4):
                    ps = psum.tile([128, 2, 2, C], f32, tag="mm", bufs=4,
                                   name=f"psvt_{b}_{c4}")
                    for it in range(2):
                        sl = slice((c4 * 2 + it) * 128, (c4 * 2 + it + 1) * 128)
                        mm(ps[:, it, :, :], x1_sb[:, sl], wvt_sb,
                           start=True, stop=True)
                    nc.vector.tensor_copy(
                        frw(vt_sb[:, c4 * 256:(c4 + 1) * 256].rearrange(
                            "p (i c) -> p i c", i=2)),
                        ps[:, :, 0, :])
                vt_sbs.append(vt_sb)
                if b == 0:
                    nc.sync.dma_start(frw(pos_sb), fr(pos))
                    nc.sync.dma_start(frw(w2t_sb), fr(w2t))

            def phase_b(b):
                # attention, software-pipelined: PE stream is att(0) att(1)
                # [s/o(0)] att(2) [s/o(1)] ... so PE never head-of-line
                # blocks on ACT's exp.
                q_sb, k_sb, vt_sb = q_sbs[b], k_sbs[b], vt_sbs[b]
                ps_s = psum.tile([128, N], f32, tag="s", bufs=1, name=f"pss_{b}")
                ps_o = psum.tile([128, N], f32, tag="o", bufs=1, name=f"pso_{b}")
                e_sb = act.tile([128, 8 * N], f32, tag="e", bufs=1, name=f"e{b}")

                # pull the exp LUT swap ahead of the first att matmul
                nc.scalar.activation(warm_sb, warm_sb, AF.Exp)

                def emit_att(jt):
                    # attT[j,i] = sum_c k[c,j] q[c,i] + sum_c q[c,j] pos[c,i]
                    # one [128,512] psum per half, each complete after 2 mms
                    sl = slice(jt * 128, (jt + 1) * 128)
                    pair = []
                    for hi, h in enumerate(HALF):
                        ps = psum.tile([128, 512], f32, tag="mm", bufs=4,
                                       name=f"psatt_{b}_{jt}_{hi}")
                        mm(ps, k_sb[:, sl], q_sb[:, h], start=True, stop=False)
                        mm(ps, q_sb[:, sl], pos_sb[:, h], start=False, stop=True)
                        pair.append(ps)
                    return pair

                def emit_so(jt):
                    sl = slice(jt * 128, (jt + 1) * 128)
                    ej = e_sb[:, jt * N:(jt + 1) * N]
                    for hi, h in enumerate(HALF):
                        # denominator: all-ones stationary -> column sums
                        # broadcast across all 128 partitions
                        mm(ps_s[:, h], ones_sb, ej[:, h],
                           start=(jt == 0), stop=(jt == 7),
                           skip_group_check=True)
                        # numerator: out[c, i] += sum_j v[c, j] e[j, i]
                        mm(ps_o[:, h], vt_sb[:, sl], ej[:, h],
                           start=(jt == 0), stop=(jt == 7),
                           skip_group_check=True)

                # lag-1 pipeline: exp(jt) consumes att(jt) while PE runs
                # att(jt+2) and s/o(jt-1); PE never waits on ACT in steady
                # state and the tail drains stall-free.
                atts = {0: emit_att(0), 1: emit_att(1)}
                for jt in range(8):
                    cur = atts.pop(jt)
                    ej = e_sb[:, jt * N:(jt + 1) * N]
                    for hi, h in enumerate(HALF):
                        nc.scalar.activation(frw(ej[:, h]), cur[hi], AF.Exp)
                    if jt + 2 <= 7:
                        atts[jt + 2] = emit_att(jt + 2)
                    if jt >= 1:
                        emit_so(jt - 1)
                emit_so(7)

                recip_sb = act.tile([128, N], f32, tag="recip", bufs=2,
                                    name=f"recip{b}")
                outn_sb = act.tile([128, N], f32, tag="outn", bufs=4,
                                   name=f"outn{b}")
                for h in HALF:
                    nc.vector.reciprocal(recip_sb[:, h], ps_s[:, h])
                    nc.vector.tensor_mul(frw(outn_sb[:, h]), ps_o[:, h],
                                         recip_sb[:, h])
                outn_sbs.append(outn_sb)

            def phase_c(b):
                for mt in range(2):
                    sl = slice(mt * 128, (mt + 1) * 128)
                    y_sb = act.tile([128, N], f32, tag="y", bufs=2,
                                    name=f"y{b}_{mt}")
                    res_sb = act.tile([128, N], f32, tag="res", bufs=2,
                                      name=f"res{b}_{mt}")
                    for hi, h in enumerate(HALF):
                        ps = psum.tile([128, 512], f32, tag="mm", bufs=4,
                                       name=f"psy_{b}_{mt}_{hi}")
                        mm(ps, w2t_sb[:, sl], outn_sbs[b][:, h],
                           start=True, stop=True)
                        silu(y_sb, ps, b2c_sb[:, mt:mt + 1], h)
                        nc.vector.tensor_add(res_sb[:, h], y_sb[:, h],
                                             x_sbs[b][:, mt, h])
                        nc.sync.dma_start(ys[b, sl, h], res_sb[:, h])

            for b in range(B_PER_CORE):
                phase_a(b)
            for b in range(B_PER_CORE):
                phase_b(b)
            for b in range(B_PER_CORE):
                phase_c(b)

    nc.compile()
    return nc


def _prepare_params(w1, bn1_g, bn1_b, bn1_m, bn1_v, wq, bq, wk, bk, wv, bv,
                    rel_h, rel_w, w2, bn2_g, bn2_b, bn2_m, bn2_v):
    f64 = np.float64
    s1 = bn1_g.astype(f64) / np.sqrt(bn1_v.astype(f64) + EPS)
    w1p = w1.astype(f64) * s1[:, None]
    t1 = bn1_b.astype(f64) - bn1_m.astype(f64) * s1
    s2 = bn2_g.astype(f64) / np.sqrt(bn2_v.astype(f64) + EPS)
    w2p = w2.astype(f64) * s2[:, None]
    t2 = bn2_b.astype(f64) - bn2_m.astype(f64) * s2
    b2 = t2 + w2p @ bv.astype(f64)
    posv = (rel_h.astype(f64) + rel_w.astype(f64)).reshape(C, N)
    f32 = np.float32
    bpk = np.stack([t1, bq.astype(f64), bk.astype(f64),
                    b2[:128], b2[128:]], axis=1)
    return {
        "w1t": np.ascontiguousarray(w1p.T, dtype=f32),
        "wqt": np.ascontiguousarray(wq.T, dtype=f32),
        "wkt": np.ascontiguousarray(wk.T, dtype=f32),
        "wvt": np.ascontiguousarray(wv.T, dtype=f32),
        "pos": np.ascontiguousarray(posv, dtype=f32),
        "w2t": np.ascontiguousarray(w2p.T, dtype=f32),
        "bpk": np.ascontiguousarray(bpk, dtype=f32),
        "onesd": np.ones((C, C), dtype=f32),
    }


def get_program(native_silu=True, use_f32r=True):
    key = ("nc", native_silu, use_f32r)
    if key not in _CACHE:
        _CACHE[key] = _build_program(native_silu, use_f32r)
    return _CACHE[key]


def make_in_maps(x, params):
    B = x.shape[0]
    per = B // N_CORES
    xr = np.ascontiguousarray(x.reshape(B, CIN, N), dtype=np.float32)
    return [
        {"xs": xr[c * per:(c + 1) * per], **params}
        for c in range(N_CORES)
    ]


def kernel(x, w1, bn1_g, bn1_b, bn1_m, bn1_v, wq, bq, wk, bk, wv, bv,
           rel_h, rel_w, w2, bn2_g, bn2_b, bn2_m, bn2_v):
    from concourse.bass_utils import run_bass_kernel_spmd

    nc = get_program()
    params = _prepare_params(w1, bn1_g, bn1_b, bn1_m, bn1_v, wq, bq, wk, bk,
                             wv, bv, rel_h, rel_w, w2, bn2_g, bn2_b, bn2_m,
                             bn2_v)
    in_maps = make_in_maps(x, params)
    res = run_bass_kernel_spmd(nc, in_maps, core_ids=list(range(N_CORES)))
    out = np.concatenate([res.results[c]["ys"] for c in range(N_CORES)], axis=0)
    return np.ascontiguousarray(out.reshape(32, CIN, 32, 32), dtype=np.float32)
